# revision 19
# baseline (speedup 1.0000x reference)
"""Multi-head causal attention (B=4, L=2048, E=1024, H=16) on 8 trn2 NeuronCores.

Sharding: (batch, head-group) grid — core c handles batch b=c//2 and heads
g=c%2 (8 heads each).  Each core computes its heads' QKV projection, causal
attention, and a partial output projection; the host sums the two partials
per batch.

v3: all matmuls bf16 (rel err ~4e-3 vs the 2e-2 gate).  bf16 halves PE power
(eliminates the K=4/8 duty-cycle throttling that dominated the f32r version),
halves DMA bytes, and enables fast weight loads.

Attention processes HEAD PAIRS: heads 2m (SBUF partitions 0-63) and 2m+1
(partitions 64-127) issue back-to-back QK matmuls as concurrent 64x128
row-tiles of the PE array (tile_position auto-derived from base partitions),
doubling QK throughput.  Scores for both heads land in one [128, 2, 512]
PSUM tile so a single ACT exp instruction covers the pair.  PV keeps the
per-head ones-column (M=65): the softmax denominator falls out of the PV
accumulation for free; its reciprocal is broadcast across partitions with a
GPSIMD partition_broadcast (no DRAM round-trip).

lq runs in 512-wide chunks (4 per pair); dense work (QKV projection chunks
2,3; output projection rows) interleaves into the ACT-paced attention as PE
heaters from a dedicated 1-bank PSUM pool so it never steals the score
buffers.  PSUM evacuation of the PV accumulators rides the scalar engine
(exp and copy share an ACT table - no reload).
"""

import numpy as np

L = 2048
E = 1024
NH = 8        # heads per core
D = 64
JQ = 512      # feature rows per core (NH*D)
LT = L // 128     # 16 l-tiles
ET = E // 128     # 8 e-tiles

_CACHE = {}


def build_nc():
    import concourse.mybir as mybir
    import concourse.tile as tile
    from concourse import bacc
    from contextlib import ExitStack

    f32 = mybir.dt.float32
    bf16 = mybir.dt.bfloat16
    Exp = mybir.ActivationFunctionType.Exp

    # Bacc (not raw Bass): its compile() legalizes multi-wait instructions
    # (move_matmul_waits_to_ldweights + generate_event_semaphores) — walrus
    # rejects >1 sync wait per instruction otherwise.
    nc = bacc.Bacc("TRN2", target_bir_lowering=False, debug=False)

    xT_d = nc.declare_dram_parameter("xT", [E, L], bf16, isOutput=False)
    wqkvT_d = nc.declare_dram_parameter("wqkvT", [E, 3 * JQ], bf16, isOutput=False)
    woT_d = nc.declare_dram_parameter("woT", [JQ, E], bf16, isOutput=False)
    diag2_d = nc.declare_dram_parameter("diag2", [128, 256], bf16, isOutput=False)
    y_d = nc.declare_dram_parameter("y", [L, E], f32, isOutput=True)

    with ExitStack() as ctx:
        tc = ctx.enter_context(tile.TileContext(nc))

        consts = ctx.enter_context(tc.tile_pool(name="consts", bufs=1))
        vaug_p = ctx.enter_context(tc.tile_pool(name="vaug", bufs=1))
        qk_p = ctx.enter_context(tc.tile_pool(name="qk", bufs=1))
        ao_p = ctx.enter_context(tc.tile_pool(name="ao", bufs=1))
        wqk_p = ctx.enter_context(tc.tile_pool(name="wqk", bufs=1))
        wo_p = ctx.enter_context(tc.tile_pool(name="wo", bufs=1))
        wv_p = ctx.enter_context(tc.tile_pool(name="wv", bufs=1))
        xT_p = ctx.enter_context(tc.tile_pool(name="xT", bufs=3))

        # PSUM: scores 2x2 banks, PV accumulators 3x1, heaters 1x1 = 8 banks
        sc_pp = ctx.enter_context(tc.tile_pool(name="scpp", bufs=2, space="PSUM"))
        pv_pp = ctx.enter_context(tc.tile_pool(name="pvpp", bufs=3, space="PSUM"))
        heat_pp = ctx.enter_context(tc.tile_pool(name="heat", bufs=1, space="PSUM"))

        pe_p = ctx.enter_context(tc.tile_pool(name="pe", bufs=3))
        aou_p = ctx.enter_context(tc.tile_pool(name="aou", bufs=3))
        rc_p = ctx.enter_context(tc.tile_pool(name="rc", bufs=3))
        rcb_p = ctx.enter_context(tc.tile_pool(name="rcb", bufs=3))
        rcd_p = ctx.enter_context(tc.tile_pool(name="rcd", bufs=3, space="DRAM"))
        y_p = ctx.enter_context(tc.tile_pool(name="y", bufs=3))

        # ---------------- head: constant + weight + x loads ----------------
        diag2_sb = consts.tile([128, 2, 128], bf16)
        # v in 4 lt-quadrant tiles: v-projection units run concurrently with
        # attention on earlier quadrants; split tiles keep the subtile
        # dependency tracking from serializing PV behind unrelated writes
        vq = [vaug_p.tile([128, 4, NH, 65], bf16, name=f"vq{q}") for q in range(4)]
        for q in range(4):
            nc.vector.memset(vq[q][:, :, :, 64:65], 1.0)

        # per-chunk q/k tiles and per-pair attention-output tiles: keeps
        # the subtile dependency tracking precise (a monolithic tile makes
        # late readers wait on the newest writer of ANY region)
        qTc = [qk_p.tile([128, 4, 512], bf16, name=f"qT{c}") for c in range(4)]
        kTc = [qk_p.tile([128, 4, 512], bf16, name=f"kT{c}") for c in range(4)]
        aoTm = [ao_p.tile([128, L], bf16, name=f"aoT{m}") for m in range(4)]

        wvT_sb = wv_p.tile([128, ET, JQ], bf16)           # 8KB/part
        wqkT_sb = wqk_p.tile([128, ET, 2 * JQ], bf16)     # 16KB/part
        woT_sb = wo_p.tile([128, 4, E], bf16)             # 8KB/part

        def half_load(dst, src, cols):
            # split a (et p) input load into two et-halves so compute can
            # start when the first half lands
            for h in range(2):
                nc.sync.dma_start(
                    out=dst[:, 4 * h:4 * h + 4, :],
                    in_=src[512 * h:512 * h + 512, cols].rearrange(
                        "(et p) l -> p et l", p=128
                    ),
                )

        xcs = [xT_p.tile([128, ET, 512], bf16, tag="xc", name=f"xc{c}") for c in range(4)]
        for h in range(2):  # interleaved so the first et-half lands first
            nc.sync.dma_start(
                out=wvT_sb[:, 4 * h:4 * h + 4, :],
                in_=wqkvT_d.ap()[512 * h:512 * h + 512, 2 * JQ:3 * JQ].rearrange(
                    "(et p) j -> p et j", p=128),
            )
            nc.sync.dma_start(
                out=xcs[0][:, 4 * h:4 * h + 4, :],
                in_=xT_d.ap()[512 * h:512 * h + 512, 0:512].rearrange(
                    "(et p) l -> p et l", p=128),
            )
        half_load(wqkT_sb, wqkvT_d.ap(), slice(0, 2 * JQ))

        def dense_ps(pp, nm):
            # heaters get the dedicated 1-bank pool; dense phases borrow the
            # (idle) 2-bank score rotation so back-to-back units pipeline
            if pp is heat_pp:
                return pp.tile([128, 512], f32, tag="hp", name=nm)
            return pp.tile([128, 2, 512], f32, tag="sc", name=nm)[:, 0, :]

        def v_unit(pp, xc, lt, i):
            ps = dense_ps(pp, "vps")
            for et in range(ET):
                nc.tensor.matmul(
                    ps,
                    lhsT=xc[:, et, i * 128:(i + 1) * 128],
                    rhs=wvT_sb[:, et, :],
                    start=(et == 0), stop=(et == ET - 1),
                )
            nc.vector.tensor_copy(
                out=vq[lt // 4][:, lt % 4, :, 0:64],
                in_=ps.rearrange("p (h d) -> p h d", h=NH),
            )

        def qk_unit(pp, xc, jt, c):
            # jt 0..3 = q j-tiles, 4..7 = k j-tiles
            ps = dense_ps(pp, "qkps")
            dst = qTc[c] if jt < 4 else kTc[c]
            for et in range(ET):
                nc.tensor.matmul(
                    ps,
                    lhsT=wqkT_sb[:, et, jt * 128:(jt + 1) * 128],
                    rhs=xc[:, et, :],
                    start=(et == 0), stop=(et == ET - 1),
                )
            nc.vector.tensor_copy(out=dst[:, jt % 4, :], in_=ps)

        def op_unit(pp, lt, ec):
            ps = dense_ps(pp, "opps")
            for jt in range(4):
                nc.tensor.matmul(
                    ps,
                    lhsT=aoTm[jt][:, lt * 128:(lt + 1) * 128],
                    rhs=woT_sb[:, jt, ec * 512:(ec + 1) * 512],
                    start=(jt == 0), stop=(jt == 3),
                )
            yt = y_p.tile([128, 512], f32, tag="y")
            nc.vector.tensor_copy(out=yt, in_=ps)
            nc.sync.dma_start(
                out=y_d.ap()[lt * 128:(lt + 1) * 128, ec * 512:(ec + 1) * 512],
                in_=yt,
            )

        # ---------------- P0a: v lt0-3 + qk chunk 0 (minimum dense prefix) ---
        nunit = 0

        def alt_pool():
            nonlocal nunit
            nunit += 1
            return heat_pp if nunit % 2 else sc_pp

        half_load(xcs[1], xT_d.ap(), slice(512, 1024))
        nc.sync.dma_start(
            out=diag2_sb, in_=diag2_d.ap().rearrange("p (a k) -> p a k", a=2)
        )
        for i in range(4):
            v_unit(alt_pool(), xcs[0], i, i)
        half_load(xcs[2], xT_d.ap(), slice(1024, 1536))
        half_load(xcs[3], xT_d.ap(), slice(1536, 2048))
        nc.sync.dma_start(
            out=woT_sb, in_=woT_d.ap().rearrange("(jt p) e -> p jt e", p=128)
        )
        for jt in range(8):
            qk_unit(alt_pool(), xcs[0], jt, 0)

        # ---------------- attention ----------------
        def attn_chunk(m, c, heaters, period, last=False):
            """Head pair m (heads 2m, 2m+1), lq chunk c (cols 512c..512c+511)."""
            ntiles = 4 * c + 4
            pv_A = pv_pp.tile([65, 512], f32, tag="pv", name="pvA")
            pv_B = pv_pp.tile([65, 512], f32, tag="pv", name="pvB")
            pvs = (pv_A, pv_B)
            pending = None

            def emit_pv(pend, last):
                pe, off, t = pend
                for hi in (0, 1):
                    nc.tensor.matmul(
                        pvs[hi][:, off:512],
                        lhsT=vq[t // 4][:, t % 4, 2 * m + hi, :],
                        rhs=pe[:, hi, off:512],
                        start=(t == 0), stop=last,
                        skip_group_check=True,
                    )

            for t in range(ntiles):
                j = t - 4 * c
                off = 128 * j if j > 0 else 0
                sc = sc_pp.tile([128, 2, 512], f32, tag="sc", name="sc")
                for hi, po in ((0, 0), (1, 64)):
                    nc.tensor.matmul(
                        sc[:, hi, off:512],
                        lhsT=kTc[t // 4][po:po + 64, m,
                                         (t % 4) * 128:(t % 4) * 128 + 128],
                        rhs=qTc[c][po:po + 64, m, off:512],
                        start=True, stop=True,
                    )
                pe = pe_p.tile([128, 2, 512], bf16, tag="pe")
                nc.scalar.activation(
                    out=pe[:, :, off:512], in_=sc[:, :, off:512], func=Exp, scale=0.125,
                )
                if j >= 0:  # diagonal block: zero lk > lq
                    nc.vector.tensor_mul(
                        out=pe[:, :, off:off + 128],
                        in0=pe[:, :, off:off + 128],
                        in1=diag2_sb,
                    )
                if pending is not None:
                    emit_pv(pending, last=False)
                pending = (pe, off, t)
                if t % period == period - 1 and heaters:
                    heaters.pop(0)()
            emit_pv(pending, last=True)

            # epilogue: evacuate PV accumulators, then reciprocal of the
            # ones-column sums via DRAM respread across 128 lanes (stride-0
            # partition reads are only legal from DRAM), normalize (bf16)
            aoU = aou_p.tile([65, 1024], f32, tag="aou")
            nc.vector.tensor_copy(out=aoU[:, 0:512], in_=pv_A)
            nc.vector.tensor_copy(out=aoU[:, 512:1024], in_=pv_B)
            rcb = rcb_p.tile([64, 1024], f32, tag="rcb")
            rcd = rcd_p.tile([1, 1024], f32, tag="rcd")
            nc.sync.dma_start(out=rcd, in_=aoU[64:65, :])
            rc8 = rc_p.tile([128, 8], f32, tag="rc8")
            nc.sync.dma_start(out=rc8, in_=rcd.rearrange("o (p k) -> (o p) k", p=128))
            nc.vector.reciprocal(out=rc8, in_=rc8)
            rcd2 = rcd_p.tile([1, 1024], f32, tag="rcd2")
            nc.sync.dma_start(out=rcd2.rearrange("o (p k) -> (o p) k", p=128), in_=rc8)
            nc.sync.dma_start(out=rcb, in_=rcd2.to_broadcast((64, 1024)))
            for hi, po in ((0, 0), (1, 64)):
                nc.vector.tensor_mul(
                    out=aoTm[m][po:po + 64, c * 512:(c + 1) * 512],
                    in0=aoU[0:64, hi * 512:hi * 512 + 512],
                    in1=rcb[:, hi * 512:hi * 512 + 512],
                )

        # Phase driver: chunk-c attention for all pairs, carrying the dense
        # units needed by LATER phases as heaters (qk chunk c+1, v quadrant
        # c+1, then the output projection).  Every phase stays PE-bound, so
        # the ACT exp stream hides under dense matmuls instead of pacing.
        def run_phase(c, heaters, period, last=False):
            for m in range(4):
                attn_chunk(m, c, heaters, period, last=last and m == 3)
            while heaters:
                heaters.pop(0)()

        run_phase(0, (
            [lambda jt=jt: qk_unit(heat_pp, xcs[1], jt, 1) for jt in range(8)]
            + [lambda i=i: v_unit(heat_pp, xcs[1], 4 + i, i) for i in range(4)]
        ), period=1)
        run_phase(1, (
            [lambda jt=jt: qk_unit(heat_pp, xcs[2], jt, 2) for jt in range(8)]
            + [lambda i=i: v_unit(heat_pp, xcs[2], 8 + i, i) for i in range(4)]
        ), period=2)
        run_phase(2, (
            [lambda jt=jt: qk_unit(heat_pp, xcs[3], jt, 3) for jt in range(8)]
            + [lambda i=i: v_unit(heat_pp, xcs[3], 12 + i, i) for i in range(4)]
        ), period=4)
        run_phase(3, (
            [lambda lt=lt, ec=ec: op_unit(heat_pp, lt, ec)
             for lt in range(12) for ec in range(2)]
        ), period=2)

        # tail: outproj rows 1536..2047
        for lt in range(12, LT):
            for ec in range(2):
                op_unit(alt_pool(), lt, ec)

    nc.compile()
    return nc


def make_in_maps(x, w_qkv, wo):
    """Host-side sharding: 8 cores = (batch b=c//2, head-group g=c%2)."""
    import ml_dtypes
    bf16 = ml_dtypes.bfloat16

    x = np.asarray(x, dtype=np.float32)
    w_qkv = np.asarray(w_qkv, dtype=np.float32)
    wo = np.asarray(wo, dtype=np.float32)
    tri = np.triu(np.ones((128, 128), np.float32))
    diag2 = np.concatenate([tri, tri], axis=1).astype(bf16)
    in_maps = []
    for c in range(8):
        b, g = c // 2, c % 2
        js = slice(g * JQ, (g + 1) * JQ)
        wq = w_qkv[0:E][js]
        wk = w_qkv[E:2 * E][js]
        wv = w_qkv[2 * E:3 * E][js]
        in_maps.append({
            "xT": np.ascontiguousarray(x[b].T).astype(bf16),
            "wqkvT": np.ascontiguousarray(np.concatenate([wq, wk, wv], 0).T).astype(bf16),
            "woT": np.ascontiguousarray(wo[:, js].T).astype(bf16),
            "diag2": diag2,
        })
    return in_maps


def _get_nc():
    if "nc" not in _CACHE:
        _CACHE["nc"] = build_nc()
    return _CACHE["nc"]


def kernel(x, mask, w_qkv, wo, _trace=False, _trace_kwargs=None):
    from concourse.bass_utils import run_bass_kernel_spmd

    nc = _get_nc()
    in_maps = make_in_maps(x, w_qkv, wo)
    res = run_bass_kernel_spmd(
        nc, in_maps, core_ids=list(range(8)),
        trace=_trace, **(_trace_kwargs or {}),
    )
    _CACHE["last_results"] = res
    y = np.stack([res.results[2 * b]["y"] + res.results[2 * b + 1]["y"] for b in range(4)])
    return y.astype(np.float32)


# revision 20
# speedup vs baseline: 1.2247x; 1.2247x over previous
"""Multi-head causal attention (B=4, L=2048, E=1024, H=16) on 8 trn2 NeuronCores.

Sharding: (batch, head-group) grid — core c handles batch b=c//2 and heads
g=c%2 (8 heads each).  Each core computes its heads' QKV projection, causal
attention, and a partial output projection; the host sums the two partials
per batch.

v3: all matmuls bf16 (rel err ~4e-3 vs the 2e-2 gate).  bf16 halves PE power
(eliminates the K=4/8 duty-cycle throttling that dominated the f32r version),
halves DMA bytes, and enables fast weight loads.

Attention processes HEAD PAIRS: heads 2m (SBUF partitions 0-63) and 2m+1
(partitions 64-127) issue back-to-back QK matmuls as concurrent 64x128
row-tiles of the PE array (tile_position auto-derived from base partitions),
doubling QK throughput.  Scores for both heads land in one [128, 2, 512]
PSUM tile so a single ACT exp instruction covers the pair.  PV keeps the
per-head ones-column (M=65): the softmax denominator falls out of the PV
accumulation for free; its reciprocal is broadcast across partitions with a
GPSIMD partition_broadcast (no DRAM round-trip).

lq runs in 512-wide chunks (4 per pair); dense work (QKV projection chunks
2,3; output projection rows) interleaves into the ACT-paced attention as PE
heaters from a dedicated 1-bank PSUM pool so it never steals the score
buffers.  PSUM evacuation of the PV accumulators rides the scalar engine
(exp and copy share an ACT table - no reload).
"""

import numpy as np

L = 2048
E = 1024
NH = 8        # heads per core
D = 64
JQ = 512      # feature rows per core (NH*D)
LT = L // 128     # 16 l-tiles
ET = E // 128     # 8 e-tiles

_CACHE = {}


def build_nc():
    import concourse.mybir as mybir
    import concourse.tile as tile
    from concourse import bacc
    from contextlib import ExitStack

    f32 = mybir.dt.float32
    bf16 = mybir.dt.bfloat16
    Exp = mybir.ActivationFunctionType.Exp

    # Bacc (not raw Bass): its compile() legalizes multi-wait instructions
    # (move_matmul_waits_to_ldweights + generate_event_semaphores) — walrus
    # rejects >1 sync wait per instruction otherwise.
    nc = bacc.Bacc("TRN2", target_bir_lowering=False, debug=False)

    xT_d = nc.declare_dram_parameter("xT", [E, L], bf16, isOutput=False)
    wqkvT_d = nc.declare_dram_parameter("wqkvT", [E, 3 * JQ], bf16, isOutput=False)
    woT_d = nc.declare_dram_parameter("woT", [JQ, E], bf16, isOutput=False)
    diag2_d = nc.declare_dram_parameter("diag2", [128, 256], bf16, isOutput=False)
    y_d = nc.declare_dram_parameter("y", [L, E], f32, isOutput=True)

    with ExitStack() as ctx:
        tc = ctx.enter_context(tile.TileContext(nc))

        consts = ctx.enter_context(tc.tile_pool(name="consts", bufs=1))
        vaug_p = ctx.enter_context(tc.tile_pool(name="vaug", bufs=1))
        qk_p = ctx.enter_context(tc.tile_pool(name="qk", bufs=1))
        ao_p = ctx.enter_context(tc.tile_pool(name="ao", bufs=1))
        wqk_p = ctx.enter_context(tc.tile_pool(name="wqk", bufs=1))
        wo_p = ctx.enter_context(tc.tile_pool(name="wo", bufs=1))
        wv_p = ctx.enter_context(tc.tile_pool(name="wv", bufs=1))
        xT_p = ctx.enter_context(tc.tile_pool(name="xT", bufs=3))

        # PSUM: scores 2x2 banks, PV accumulators 3x1, heaters 1x1 = 8 banks
        sc_pp = ctx.enter_context(tc.tile_pool(name="scpp", bufs=2, space="PSUM"))
        pv_pp = ctx.enter_context(tc.tile_pool(name="pvpp", bufs=3, space="PSUM"))
        heat_pp = ctx.enter_context(tc.tile_pool(name="heat", bufs=1, space="PSUM"))

        pe_p = ctx.enter_context(tc.tile_pool(name="pe", bufs=3))
        aou_p = ctx.enter_context(tc.tile_pool(name="aou", bufs=3))
        rc_p = ctx.enter_context(tc.tile_pool(name="rc", bufs=3))
        rcb_p = ctx.enter_context(tc.tile_pool(name="rcb", bufs=3))
        rcd_p = ctx.enter_context(tc.tile_pool(name="rcd", bufs=3, space="DRAM"))
        y_p = ctx.enter_context(tc.tile_pool(name="y", bufs=3))

        # ---------------- head: constant + weight + x loads ----------------
        diag2_sb = consts.tile([128, 2, 128], bf16)
        # v in 4 lt-quadrant tiles: v-projection units run concurrently with
        # attention on earlier quadrants; split tiles keep the subtile
        # dependency tracking from serializing PV behind unrelated writes
        vq = [vaug_p.tile([128, 4, NH, 65], bf16, name=f"vq{q}") for q in range(4)]
        for q in range(4):
            nc.vector.memset(vq[q][:, :, :, 64:65], 1.0)

        # per-chunk q/k tiles and per-pair attention-output tiles: keeps
        # the subtile dependency tracking precise (a monolithic tile makes
        # late readers wait on the newest writer of ANY region)
        qTc = [qk_p.tile([128, 4, 512], bf16, name=f"qT{c}") for c in range(4)]
        kTc = [qk_p.tile([128, 4, 512], bf16, name=f"kT{c}") for c in range(4)]
        aoTm = [ao_p.tile([128, L], bf16, name=f"aoT{m}") for m in range(4)]

        wvT_sb = wv_p.tile([128, ET, JQ], bf16)           # 8KB/part
        wqkT_sb = wqk_p.tile([128, ET, 2 * JQ], bf16)     # 16KB/part
        woT_sb = wo_p.tile([128, 4, E], bf16)             # 8KB/part

        def half_load(dst, src, cols):
            # split a (et p) input load into two et-halves so compute can
            # start when the first half lands
            for h in range(2):
                nc.sync.dma_start(
                    out=dst[:, 4 * h:4 * h + 4, :],
                    in_=src[512 * h:512 * h + 512, cols].rearrange(
                        "(et p) l -> p et l", p=128
                    ),
                )

        xcs = [xT_p.tile([128, ET, 512], bf16, tag="xc", name=f"xc{c}") for c in range(4)]
        for h in range(2):  # interleaved so the first et-half lands first
            nc.sync.dma_start(
                out=wvT_sb[:, 4 * h:4 * h + 4, :],
                in_=wqkvT_d.ap()[512 * h:512 * h + 512, 2 * JQ:3 * JQ].rearrange(
                    "(et p) j -> p et j", p=128),
            )
            nc.sync.dma_start(
                out=xcs[0][:, 4 * h:4 * h + 4, :],
                in_=xT_d.ap()[512 * h:512 * h + 512, 0:512].rearrange(
                    "(et p) l -> p et l", p=128),
            )
        half_load(wqkT_sb, wqkvT_d.ap(), slice(0, 2 * JQ))

        def dense_ps(pp, nm):
            # heaters get the dedicated 1-bank pool; dense phases borrow the
            # (idle) 2-bank score rotation so back-to-back units pipeline
            if pp is heat_pp:
                return pp.tile([128, 512], f32, tag="hp", name=nm)
            return pp.tile([128, 2, 512], f32, tag="sc", name=nm)[:, 0, :]

        def v_unit(pp, xc, lt, i):
            ps = dense_ps(pp, "vps")
            for et in range(ET):
                nc.tensor.matmul(
                    ps,
                    lhsT=xc[:, et, i * 128:(i + 1) * 128],
                    rhs=wvT_sb[:, et, :],
                    start=(et == 0), stop=(et == ET - 1),
                )
            nc.vector.tensor_copy(
                out=vq[lt // 4][:, lt % 4, :, 0:64],
                in_=ps.rearrange("p (h d) -> p h d", h=NH),
            )

        def qk_unit(pp, xc, jt, c):
            # jt 0..3 = q j-tiles, 4..7 = k j-tiles
            ps = dense_ps(pp, "qkps")
            dst = qTc[c] if jt < 4 else kTc[c]
            for et in range(ET):
                nc.tensor.matmul(
                    ps,
                    lhsT=wqkT_sb[:, et, jt * 128:(jt + 1) * 128],
                    rhs=xc[:, et, :],
                    start=(et == 0), stop=(et == ET - 1),
                )
            nc.vector.tensor_copy(out=dst[:, jt % 4, :], in_=ps)

        def op_unit(pp, lt, ec):
            ps = dense_ps(pp, "opps")
            for jt in range(4):
                nc.tensor.matmul(
                    ps,
                    lhsT=aoTm[jt][:, lt * 128:(lt + 1) * 128],
                    rhs=woT_sb[:, jt, ec * 512:(ec + 1) * 512],
                    start=(jt == 0), stop=(jt == 3),
                )
            yt = y_p.tile([128, 512], f32, tag="y")
            nc.vector.tensor_copy(out=yt, in_=ps)
            nc.sync.dma_start(
                out=y_d.ap()[lt * 128:(lt + 1) * 128, ec * 512:(ec + 1) * 512],
                in_=yt,
            )

        # ---------------- P0a: v lt0-3 + qk chunk 0 (minimum dense prefix) ---
        nunit = 0

        def alt_pool():
            nonlocal nunit
            nunit += 1
            return heat_pp if nunit % 2 else sc_pp

        half_load(xcs[1], xT_d.ap(), slice(512, 1024))
        nc.sync.dma_start(
            out=diag2_sb, in_=diag2_d.ap().rearrange("p (a k) -> p a k", a=2)
        )
        for i in range(4):
            v_unit(alt_pool(), xcs[0], i, i)
        half_load(xcs[2], xT_d.ap(), slice(1024, 1536))
        half_load(xcs[3], xT_d.ap(), slice(1536, 2048))
        nc.sync.dma_start(
            out=woT_sb, in_=woT_d.ap().rearrange("(jt p) e -> p jt e", p=128)
        )
        for jt in range(8):
            qk_unit(alt_pool(), xcs[0], jt, 0)

        # ---------------- attention ----------------
        def attn_chunk(m, c, heaters, period, last=False):
            """Head pair m (heads 2m, 2m+1), lq chunk c (cols 512c..512c+511)."""
            ntiles = 4 * c + 4
            pv_A = pv_pp.tile([65, 512], f32, tag="pv", name="pvA")
            pv_B = pv_pp.tile([65, 512], f32, tag="pv", name="pvB")
            pvs = (pv_A, pv_B)
            pending = None

            def emit_pv(pend, last):
                pe, off, t = pend
                for hi in (0, 1):
                    nc.tensor.matmul(
                        pvs[hi][:, off:512],
                        lhsT=vq[t // 4][:, t % 4, 2 * m + hi, :],
                        rhs=pe[:, hi, off:512],
                        start=(t == 0), stop=last,
                        skip_group_check=True,
                    )

            for t in range(ntiles):
                j = t - 4 * c
                off = 128 * j if j > 0 else 0
                sc = sc_pp.tile([128, 2, 512], f32, tag="sc", name="sc")
                for hi, po in ((0, 0), (1, 64)):
                    nc.tensor.matmul(
                        sc[:, hi, off:512],
                        lhsT=kTc[t // 4][po:po + 64, m,
                                         (t % 4) * 128:(t % 4) * 128 + 128],
                        rhs=qTc[c][po:po + 64, m, off:512],
                        start=True, stop=True,
                    )
                pe = pe_p.tile([128, 2, 512], bf16, tag="pe")
                nc.scalar.activation(
                    out=pe[:, :, off:512], in_=sc[:, :, off:512], func=Exp, scale=0.125,
                )
                if j >= 0:  # diagonal block: zero lk > lq
                    nc.vector.tensor_mul(
                        out=pe[:, :, off:off + 128],
                        in0=pe[:, :, off:off + 128],
                        in1=diag2_sb,
                    )
                if pending is not None:
                    emit_pv(pending, last=False)
                pending = (pe, off, t)
                if t % period == period - 1 and heaters:
                    heaters.pop(0)()
            emit_pv(pending, last=True)

            # epilogue: evacuate PV accumulators, then reciprocal of the
            # ones-column sums via DRAM respread across 128 lanes (stride-0
            # partition reads are only legal from DRAM), normalize (bf16)
            aoU = aou_p.tile([65, 1024], f32, tag="aou")
            nc.vector.tensor_copy(out=aoU[:, 0:512], in_=pv_A)
            nc.vector.tensor_copy(out=aoU[:, 512:1024], in_=pv_B)
            rcb = rcb_p.tile([64, 1024], f32, tag="rcb")
            rcd = rcd_p.tile([1, 1024], f32, tag="rcd")
            nc.sync.dma_start(out=rcd, in_=aoU[64:65, :])
            rc8 = rc_p.tile([128, 8], f32, tag="rc8")
            nc.sync.dma_start(out=rc8, in_=rcd.rearrange("o (p k) -> (o p) k", p=128))
            nc.vector.reciprocal(out=rc8, in_=rc8)
            rcd2 = rcd_p.tile([1, 1024], f32, tag="rcd2")
            nc.sync.dma_start(out=rcd2.rearrange("o (p k) -> (o p) k", p=128), in_=rc8)
            nc.sync.dma_start(out=rcb, in_=rcd2.to_broadcast((64, 1024)))
            for hi, po in ((0, 0), (1, 64)):
                nc.vector.tensor_mul(
                    out=aoTm[m][po:po + 64, c * 512:(c + 1) * 512],
                    in0=aoU[0:64, hi * 512:hi * 512 + 512],
                    in1=rcb[:, hi * 512:hi * 512 + 512],
                )

        # Phase driver: chunk-c attention for all pairs, carrying the dense
        # units needed by LATER phases as heaters (qk chunk c+1, v quadrant
        # c+1, then the output projection).  Every phase stays PE-bound, so
        # the ACT exp stream hides under dense matmuls instead of pacing.
        def run_phase(c, heaters, period, last=False):
            for m in range(4):
                attn_chunk(m, c, heaters, period, last=last and m == 3)
            while heaters:
                heaters.pop(0)()

        run_phase(0, (
            [lambda jt=jt: qk_unit(heat_pp, xcs[1], jt, 1) for jt in range(8)]
            + [lambda i=i: v_unit(heat_pp, xcs[1], 4 + i, i) for i in range(4)]
        ), period=1)
        run_phase(1, (
            [lambda jt=jt: qk_unit(heat_pp, xcs[2], jt, 2) for jt in range(8)]
            + [lambda i=i: v_unit(heat_pp, xcs[2], 8 + i, i) for i in range(4)]
        ), period=2)
        run_phase(2, (
            [lambda jt=jt: qk_unit(heat_pp, xcs[3], jt, 3) for jt in range(8)]
            + [lambda i=i: v_unit(heat_pp, xcs[3], 12 + i, i) for i in range(4)]
        ), period=4)
        # period 4: 16 of the 24 outproj units interleave into the (PE-bound)
        # c3 phase; the remaining 8 drain right after the last pair's PV —
        # i.e. inside the final denominator-chain window that was previously
        # pure PE idle (and dropped the clock to K=4/8 for the tail).
        run_phase(3, (
            [lambda lt=lt, ec=ec: op_unit(heat_pp, lt, ec)
             for lt in range(12) for ec in range(2)]
        ), period=4)

        # tail: outproj rows 1536..2047
        for lt in range(12, LT):
            for ec in range(2):
                op_unit(alt_pool(), lt, ec)

    nc.compile()
    return nc


def make_in_maps(x, w_qkv, wo):
    """Host-side sharding: 8 cores = (batch b=c//2, head-group g=c%2)."""
    import ml_dtypes
    bf16 = ml_dtypes.bfloat16

    x = np.asarray(x, dtype=np.float32)
    w_qkv = np.asarray(w_qkv, dtype=np.float32)
    wo = np.asarray(wo, dtype=np.float32)
    tri = np.triu(np.ones((128, 128), np.float32))
    diag2 = np.concatenate([tri, tri], axis=1).astype(bf16)
    in_maps = []
    for c in range(8):
        b, g = c // 2, c % 2
        js = slice(g * JQ, (g + 1) * JQ)
        wq = w_qkv[0:E][js]
        wk = w_qkv[E:2 * E][js]
        wv = w_qkv[2 * E:3 * E][js]
        in_maps.append({
            "xT": np.ascontiguousarray(x[b].T).astype(bf16),
            "wqkvT": np.ascontiguousarray(np.concatenate([wq, wk, wv], 0).T).astype(bf16),
            "woT": np.ascontiguousarray(wo[:, js].T).astype(bf16),
            "diag2": diag2,
        })
    return in_maps


def _get_nc():
    if "nc" not in _CACHE:
        _CACHE["nc"] = build_nc()
    return _CACHE["nc"]


def kernel(x, mask, w_qkv, wo, _trace=False, _trace_kwargs=None):
    from concourse.bass_utils import run_bass_kernel_spmd

    nc = _get_nc()
    in_maps = make_in_maps(x, w_qkv, wo)
    res = run_bass_kernel_spmd(
        nc, in_maps, core_ids=list(range(8)),
        trace=_trace, **(_trace_kwargs or {}),
    )
    _CACHE["last_results"] = res
    y = np.stack([res.results[2 * b]["y"] + res.results[2 * b + 1]["y"] for b in range(4)])
    return y.astype(np.float32)


# revision 21
# speedup vs baseline: 1.2247x; 1.0000x over previous
"""Multi-head causal attention (B=4, L=2048, E=1024, H=16) on 8 trn2 NeuronCores.

Sharding: (batch, head-group) grid — core c handles batch b=c//2 and heads
g=c%2 (8 heads each).  Each core computes its heads' QKV projection, causal
attention, and a partial output projection; the host sums the two partials
per batch.

v3: all matmuls bf16 (rel err ~4e-3 vs the 2e-2 gate).  bf16 halves PE power
(eliminates the K=4/8 duty-cycle throttling that dominated the f32r version),
halves DMA bytes, and enables fast weight loads.

Attention processes HEAD PAIRS: heads 2m (SBUF partitions 0-63) and 2m+1
(partitions 64-127) issue back-to-back QK matmuls as concurrent 64x128
row-tiles of the PE array (tile_position auto-derived from base partitions),
doubling QK throughput.  Scores for both heads land in one [128, 2, 512]
PSUM tile so a single ACT exp instruction covers the pair.  PV keeps the
per-head ones-column (M=65): the softmax denominator falls out of the PV
accumulation for free; its reciprocal is broadcast across partitions with a
GPSIMD partition_broadcast (no DRAM round-trip).

lq runs in 512-wide chunks (4 per pair); dense work (QKV projection chunks
2,3; output projection rows) interleaves into the ACT-paced attention as PE
heaters from a dedicated 1-bank PSUM pool so it never steals the score
buffers.  PSUM evacuation of the PV accumulators rides the scalar engine
(exp and copy share an ACT table - no reload).
"""

import numpy as np

L = 2048
E = 1024
NH = 8        # heads per core
D = 64
JQ = 512      # feature rows per core (NH*D)
LT = L // 128     # 16 l-tiles
ET = E // 128     # 8 e-tiles

_CACHE = {}


def build_nc():
    import concourse.mybir as mybir
    import concourse.tile as tile
    from concourse import bacc
    from contextlib import ExitStack

    f32 = mybir.dt.float32
    bf16 = mybir.dt.bfloat16
    Exp = mybir.ActivationFunctionType.Exp

    # Bacc (not raw Bass): its compile() legalizes multi-wait instructions
    # (move_matmul_waits_to_ldweights + generate_event_semaphores) — walrus
    # rejects >1 sync wait per instruction otherwise.
    nc = bacc.Bacc("TRN2", target_bir_lowering=False, debug=False)

    xT_d = nc.declare_dram_parameter("xT", [E, L], bf16, isOutput=False)
    wqkvT_d = nc.declare_dram_parameter("wqkvT", [E, 3 * JQ], bf16, isOutput=False)
    woT_d = nc.declare_dram_parameter("woT", [JQ, E], bf16, isOutput=False)
    diag2_d = nc.declare_dram_parameter("diag2", [128, 256], bf16, isOutput=False)
    y_d = nc.declare_dram_parameter("y", [L, E], bf16, isOutput=True)

    with ExitStack() as ctx:
        tc = ctx.enter_context(tile.TileContext(nc))

        consts = ctx.enter_context(tc.tile_pool(name="consts", bufs=1))
        vaug_p = ctx.enter_context(tc.tile_pool(name="vaug", bufs=1))
        qk_p = ctx.enter_context(tc.tile_pool(name="qk", bufs=1))
        ao_p = ctx.enter_context(tc.tile_pool(name="ao", bufs=1))
        wqk_p = ctx.enter_context(tc.tile_pool(name="wqk", bufs=1))
        wo_p = ctx.enter_context(tc.tile_pool(name="wo", bufs=1))
        wv_p = ctx.enter_context(tc.tile_pool(name="wv", bufs=1))
        xT_p = ctx.enter_context(tc.tile_pool(name="xT", bufs=3))

        # PSUM: scores 2x2 banks, PV accumulators 3x1, heaters 1x1 = 8 banks
        sc_pp = ctx.enter_context(tc.tile_pool(name="scpp", bufs=2, space="PSUM"))
        pv_pp = ctx.enter_context(tc.tile_pool(name="pvpp", bufs=3, space="PSUM"))
        heat_pp = ctx.enter_context(tc.tile_pool(name="heat", bufs=1, space="PSUM"))

        pe_p = ctx.enter_context(tc.tile_pool(name="pe", bufs=3))
        aou_p = ctx.enter_context(tc.tile_pool(name="aou", bufs=3))
        rc_p = ctx.enter_context(tc.tile_pool(name="rc", bufs=3))
        rcb_p = ctx.enter_context(tc.tile_pool(name="rcb", bufs=3))
        rcd_p = ctx.enter_context(tc.tile_pool(name="rcd", bufs=3, space="DRAM"))
        y_p = ctx.enter_context(tc.tile_pool(name="y", bufs=3))

        # ---------------- head: constant + weight + x loads ----------------
        diag2_sb = consts.tile([128, 2, 128], bf16)
        # v in 4 lt-quadrant tiles: v-projection units run concurrently with
        # attention on earlier quadrants; split tiles keep the subtile
        # dependency tracking from serializing PV behind unrelated writes
        vq = [vaug_p.tile([128, 4, NH, 65], bf16, name=f"vq{q}") for q in range(4)]
        for q in range(4):
            nc.vector.memset(vq[q][:, :, :, 64:65], 1.0)

        # per-chunk q/k tiles and per-pair attention-output tiles: keeps
        # the subtile dependency tracking precise (a monolithic tile makes
        # late readers wait on the newest writer of ANY region)
        qTc = [qk_p.tile([128, 4, 512], bf16, name=f"qT{c}") for c in range(4)]
        kTc = [qk_p.tile([128, 4, 512], bf16, name=f"kT{c}") for c in range(4)]
        aoTm = [ao_p.tile([128, L], bf16, name=f"aoT{m}") for m in range(4)]

        wvT_sb = wv_p.tile([128, ET, JQ], bf16)           # 8KB/part
        wqkT_sb = wqk_p.tile([128, ET, 2 * JQ], bf16)     # 16KB/part
        woT_sb = wo_p.tile([128, 4, E], bf16)             # 8KB/part

        def half_load(dst, src, cols):
            # split a (et p) input load into two et-halves so compute can
            # start when the first half lands
            for h in range(2):
                nc.sync.dma_start(
                    out=dst[:, 4 * h:4 * h + 4, :],
                    in_=src[512 * h:512 * h + 512, cols].rearrange(
                        "(et p) l -> p et l", p=128
                    ),
                )

        xcs = [xT_p.tile([128, ET, 512], bf16, tag="xc", name=f"xc{c}") for c in range(4)]
        for h in range(2):  # interleaved so the first et-half lands first
            nc.sync.dma_start(
                out=wvT_sb[:, 4 * h:4 * h + 4, :],
                in_=wqkvT_d.ap()[512 * h:512 * h + 512, 2 * JQ:3 * JQ].rearrange(
                    "(et p) j -> p et j", p=128),
            )
            nc.sync.dma_start(
                out=xcs[0][:, 4 * h:4 * h + 4, :],
                in_=xT_d.ap()[512 * h:512 * h + 512, 0:512].rearrange(
                    "(et p) l -> p et l", p=128),
            )
        half_load(wqkT_sb, wqkvT_d.ap(), slice(0, 2 * JQ))

        def dense_ps(pp, nm):
            # heaters get the dedicated 1-bank pool; dense phases borrow the
            # (idle) 2-bank score rotation so back-to-back units pipeline
            if pp is heat_pp:
                return pp.tile([128, 512], f32, tag="hp", name=nm)
            return pp.tile([128, 2, 512], f32, tag="sc", name=nm)[:, 0, :]

        def v_unit(pp, xc, lt, i):
            ps = dense_ps(pp, "vps")
            for et in range(ET):
                nc.tensor.matmul(
                    ps,
                    lhsT=xc[:, et, i * 128:(i + 1) * 128],
                    rhs=wvT_sb[:, et, :],
                    start=(et == 0), stop=(et == ET - 1),
                )
            nc.vector.tensor_copy(
                out=vq[lt // 4][:, lt % 4, :, 0:64],
                in_=ps.rearrange("p (h d) -> p h d", h=NH),
            )

        def qk_unit(pp, xc, jt, c):
            # jt 0..3 = q j-tiles, 4..7 = k j-tiles
            ps = dense_ps(pp, "qkps")
            dst = qTc[c] if jt < 4 else kTc[c]
            for et in range(ET):
                nc.tensor.matmul(
                    ps,
                    lhsT=wqkT_sb[:, et, jt * 128:(jt + 1) * 128],
                    rhs=xc[:, et, :],
                    start=(et == 0), stop=(et == ET - 1),
                )
            nc.vector.tensor_copy(out=dst[:, jt % 4, :], in_=ps)

        def op_unit(pp, lt, ec):
            ps = dense_ps(pp, "opps")
            for jt in range(4):
                nc.tensor.matmul(
                    ps,
                    lhsT=aoTm[jt][:, lt * 128:(lt + 1) * 128],
                    rhs=woT_sb[:, jt, ec * 512:(ec + 1) * 512],
                    start=(jt == 0), stop=(jt == 3),
                )
            yt = y_p.tile([128, 512], bf16, tag="y")
            nc.vector.tensor_copy(out=yt, in_=ps)
            nc.sync.dma_start(
                out=y_d.ap()[lt * 128:(lt + 1) * 128, ec * 512:(ec + 1) * 512],
                in_=yt,
            )

        # ---------------- P0a: v lt0-3 + qk chunk 0 (minimum dense prefix) ---
        nunit = 0

        def alt_pool():
            nonlocal nunit
            nunit += 1
            return heat_pp if nunit % 2 else sc_pp

        half_load(xcs[1], xT_d.ap(), slice(512, 1024))
        nc.sync.dma_start(
            out=diag2_sb, in_=diag2_d.ap().rearrange("p (a k) -> p a k", a=2)
        )
        for i in range(4):
            v_unit(alt_pool(), xcs[0], i, i)
        half_load(xcs[2], xT_d.ap(), slice(1024, 1536))
        half_load(xcs[3], xT_d.ap(), slice(1536, 2048))
        nc.sync.dma_start(
            out=woT_sb, in_=woT_d.ap().rearrange("(jt p) e -> p jt e", p=128)
        )
        for jt in range(8):
            qk_unit(alt_pool(), xcs[0], jt, 0)

        # ---------------- attention ----------------
        def attn_chunk(m, c, heaters, period, last=False):
            """Head pair m (heads 2m, 2m+1), lq chunk c (cols 512c..512c+511)."""
            ntiles = 4 * c + 4
            pv_A = pv_pp.tile([65, 512], f32, tag="pv", name="pvA")
            pv_B = pv_pp.tile([65, 512], f32, tag="pv", name="pvB")
            pvs = (pv_A, pv_B)
            pending = None

            def emit_pv(pend, last):
                pe, off, t = pend
                for hi in (0, 1):
                    nc.tensor.matmul(
                        pvs[hi][:, off:512],
                        lhsT=vq[t // 4][:, t % 4, 2 * m + hi, :],
                        rhs=pe[:, hi, off:512],
                        start=(t == 0), stop=last,
                        skip_group_check=True,
                    )

            for t in range(ntiles):
                j = t - 4 * c
                off = 128 * j if j > 0 else 0
                sc = sc_pp.tile([128, 2, 512], f32, tag="sc", name="sc")
                for hi, po in ((0, 0), (1, 64)):
                    nc.tensor.matmul(
                        sc[:, hi, off:512],
                        lhsT=kTc[t // 4][po:po + 64, m,
                                         (t % 4) * 128:(t % 4) * 128 + 128],
                        rhs=qTc[c][po:po + 64, m, off:512],
                        start=True, stop=True,
                    )
                pe = pe_p.tile([128, 2, 512], bf16, tag="pe")
                nc.scalar.activation(
                    out=pe[:, :, off:512], in_=sc[:, :, off:512], func=Exp, scale=0.125,
                )
                if j >= 0:  # diagonal block: zero lk > lq
                    nc.vector.tensor_mul(
                        out=pe[:, :, off:off + 128],
                        in0=pe[:, :, off:off + 128],
                        in1=diag2_sb,
                    )
                if pending is not None:
                    emit_pv(pending, last=False)
                pending = (pe, off, t)
                if t % period == period - 1 and heaters:
                    heaters.pop(0)()
            emit_pv(pending, last=True)

            # epilogue: evacuate PV accumulators, then reciprocal of the
            # ones-column sums via DRAM respread across 128 lanes (stride-0
            # partition reads are only legal from DRAM), normalize (bf16)
            aoU = aou_p.tile([65, 1024], f32, tag="aou")
            nc.vector.tensor_copy(out=aoU[:, 0:512], in_=pv_A)
            nc.vector.tensor_copy(out=aoU[:, 512:1024], in_=pv_B)
            rcb = rcb_p.tile([64, 1024], f32, tag="rcb")
            rc8 = rc_p.tile([128, 8], f32, tag="rc8")
            nc.sync.dma_start(
                out=rc8, in_=aoU[64:65, :].rearrange("o (p k) -> o p k", p=128)
            )
            nc.vector.reciprocal(out=rc8, in_=rc8)
            rcd2 = rcd_p.tile([1, 1024], f32, tag="rcd2")
            nc.sync.dma_start(out=rcd2.rearrange("o (p k) -> (o p) k", p=128), in_=rc8)
            nc.sync.dma_start(out=rcb, in_=rcd2.to_broadcast((64, 1024)))
            for hi, po in ((0, 0), (1, 64)):
                nc.vector.tensor_mul(
                    out=aoTm[m][po:po + 64, c * 512:(c + 1) * 512],
                    in0=aoU[0:64, hi * 512:hi * 512 + 512],
                    in1=rcb[:, hi * 512:hi * 512 + 512],
                )

        # Phase driver: chunk-c attention for all pairs, carrying the dense
        # units needed by LATER phases as heaters (qk chunk c+1, v quadrant
        # c+1, then the output projection).  Every phase stays PE-bound, so
        # the ACT exp stream hides under dense matmuls instead of pacing.
        def run_phase(c, heaters, period, last=False):
            for m in range(4):
                attn_chunk(m, c, heaters, period, last=last and m == 3)
            while heaters:
                heaters.pop(0)()

        run_phase(0, (
            [lambda jt=jt: qk_unit(heat_pp, xcs[1], jt, 1) for jt in range(8)]
            + [lambda i=i: v_unit(heat_pp, xcs[1], 4 + i, i) for i in range(4)]
        ), period=1)
        run_phase(1, (
            [lambda jt=jt: qk_unit(heat_pp, xcs[2], jt, 2) for jt in range(8)]
            + [lambda i=i: v_unit(heat_pp, xcs[2], 8 + i, i) for i in range(4)]
        ), period=2)
        run_phase(2, (
            [lambda jt=jt: qk_unit(heat_pp, xcs[3], jt, 3) for jt in range(8)]
            + [lambda i=i: v_unit(heat_pp, xcs[3], 12 + i, i) for i in range(4)]
        ), period=4)
        # period 4: 16 of the 24 outproj units interleave into the (PE-bound)
        # c3 phase; the remaining 8 drain right after the last pair's PV —
        # i.e. inside the final denominator-chain window that was previously
        # pure PE idle (and dropped the clock to K=4/8 for the tail).
        run_phase(3, (
            [lambda lt=lt, ec=ec: op_unit(heat_pp, lt, ec)
             for lt in range(12) for ec in range(2)]
        ), period=4)

        # tail: outproj rows 1536..2047
        for lt in range(12, LT):
            for ec in range(2):
                op_unit(alt_pool(), lt, ec)

    nc.compile()
    return nc


def make_in_maps(x, w_qkv, wo):
    """Host-side sharding: 8 cores = (batch b=c//2, head-group g=c%2)."""
    import ml_dtypes
    bf16 = ml_dtypes.bfloat16

    x = np.asarray(x, dtype=np.float32)
    w_qkv = np.asarray(w_qkv, dtype=np.float32)
    wo = np.asarray(wo, dtype=np.float32)
    tri = np.triu(np.ones((128, 128), np.float32))
    diag2 = np.concatenate([tri, tri], axis=1).astype(bf16)
    in_maps = []
    for c in range(8):
        b, g = c // 2, c % 2
        js = slice(g * JQ, (g + 1) * JQ)
        wq = w_qkv[0:E][js]
        wk = w_qkv[E:2 * E][js]
        wv = w_qkv[2 * E:3 * E][js]
        in_maps.append({
            "xT": np.ascontiguousarray(x[b].T).astype(bf16),
            "wqkvT": np.ascontiguousarray(np.concatenate([wq, wk, wv], 0).T).astype(bf16),
            "woT": np.ascontiguousarray(wo[:, js].T).astype(bf16),
            "diag2": diag2,
        })
    return in_maps


def _get_nc():
    if "nc" not in _CACHE:
        _CACHE["nc"] = build_nc()
    return _CACHE["nc"]


def kernel(x, mask, w_qkv, wo, _trace=False, _trace_kwargs=None):
    from concourse.bass_utils import run_bass_kernel_spmd

    nc = _get_nc()
    in_maps = make_in_maps(x, w_qkv, wo)
    res = run_bass_kernel_spmd(
        nc, in_maps, core_ids=list(range(8)),
        trace=_trace, **(_trace_kwargs or {}),
    )
    _CACHE["last_results"] = res
    y = np.stack([
        res.results[2 * b]["y"].astype(np.float32)
        + res.results[2 * b + 1]["y"].astype(np.float32)
        for b in range(4)
    ])
    return y


# revision 23
# speedup vs baseline: 1.2388x; 1.0115x over previous
"""Multi-head causal attention (B=4, L=2048, E=1024, H=16) on 8 trn2 NeuronCores.

Sharding: (batch, head-group) grid — core c handles batch b=c//2 and heads
g=c%2 (8 heads each).  Each core computes its heads' QKV projection, causal
attention, and a partial output projection; the host sums the two partials
per batch.

v3: all matmuls bf16 (rel err ~4e-3 vs the 2e-2 gate).  bf16 halves PE power
(eliminates the K=4/8 duty-cycle throttling that dominated the f32r version),
halves DMA bytes, and enables fast weight loads.

Attention processes HEAD PAIRS: heads 2m (SBUF partitions 0-63) and 2m+1
(partitions 64-127) issue back-to-back QK matmuls as concurrent 64x128
row-tiles of the PE array (tile_position auto-derived from base partitions),
doubling QK throughput.  Scores for both heads land in one [128, 2, 512]
PSUM tile so a single ACT exp instruction covers the pair.  PV keeps the
per-head ones-column (M=65): the softmax denominator falls out of the PV
accumulation for free; its reciprocal is broadcast across partitions with a
GPSIMD partition_broadcast (no DRAM round-trip).

lq runs in 512-wide chunks (4 per pair); dense work (QKV projection chunks
2,3; output projection rows) interleaves into the ACT-paced attention as PE
heaters from a dedicated 1-bank PSUM pool so it never steals the score
buffers.  PSUM evacuation of the PV accumulators rides the scalar engine
(exp and copy share an ACT table - no reload).
"""

import numpy as np

L = 2048
E = 1024
NH = 8        # heads per core
D = 64
JQ = 512      # feature rows per core (NH*D)
LT = L // 128     # 16 l-tiles
ET = E // 128     # 8 e-tiles

_CACHE = {}


def build_nc():
    import concourse.mybir as mybir
    import concourse.tile as tile
    from concourse import bacc
    from contextlib import ExitStack

    f32 = mybir.dt.float32
    bf16 = mybir.dt.bfloat16
    Exp = mybir.ActivationFunctionType.Exp

    # Bacc (not raw Bass): its compile() legalizes multi-wait instructions
    # (move_matmul_waits_to_ldweights + generate_event_semaphores) — walrus
    # rejects >1 sync wait per instruction otherwise.
    nc = bacc.Bacc("TRN2", target_bir_lowering=False, debug=False)

    xT_d = nc.declare_dram_parameter("xT", [E, L], bf16, isOutput=False)
    wqkvT_d = nc.declare_dram_parameter("wqkvT", [E, 3 * JQ], bf16, isOutput=False)
    woT_d = nc.declare_dram_parameter("woT", [JQ, E], bf16, isOutput=False)
    diag2_d = nc.declare_dram_parameter("diag2", [128, 256], bf16, isOutput=False)
    y_d = nc.declare_dram_parameter("y", [L, E], f32, isOutput=True)

    with ExitStack() as ctx:
        tc = ctx.enter_context(tile.TileContext(nc))

        consts = ctx.enter_context(tc.tile_pool(name="consts", bufs=1))
        vaug_p = ctx.enter_context(tc.tile_pool(name="vaug", bufs=1))
        qk_p = ctx.enter_context(tc.tile_pool(name="qk", bufs=1))
        ao_p = ctx.enter_context(tc.tile_pool(name="ao", bufs=1))
        wqk_p = ctx.enter_context(tc.tile_pool(name="wqk", bufs=1))
        wo_p = ctx.enter_context(tc.tile_pool(name="wo", bufs=1))
        wv_p = ctx.enter_context(tc.tile_pool(name="wv", bufs=1))
        xT_p = ctx.enter_context(tc.tile_pool(name="xT", bufs=3))

        # PSUM: scores 2x2 banks, PV accumulators 3x1, heaters 1x1 = 8 banks
        sc_pp = ctx.enter_context(tc.tile_pool(name="scpp", bufs=2, space="PSUM"))
        pv_pp = ctx.enter_context(tc.tile_pool(name="pvpp", bufs=3, space="PSUM"))
        heat_pp = ctx.enter_context(tc.tile_pool(name="heat", bufs=1, space="PSUM"))

        pe_p = ctx.enter_context(tc.tile_pool(name="pe", bufs=4))
        aou_p = ctx.enter_context(tc.tile_pool(name="aou", bufs=4))
        rc_p = ctx.enter_context(tc.tile_pool(name="rc", bufs=3))
        rcb_p = ctx.enter_context(tc.tile_pool(name="rcb", bufs=4))
        rcd_p = ctx.enter_context(tc.tile_pool(name="rcd", bufs=3, space="DRAM"))
        y_p = ctx.enter_context(tc.tile_pool(name="y", bufs=4))

        # ---------------- head: constant + weight + x loads ----------------
        diag2_sb = consts.tile([128, 2, 128], bf16)
        # v in 4 lt-quadrant tiles: v-projection units run concurrently with
        # attention on earlier quadrants; split tiles keep the subtile
        # dependency tracking from serializing PV behind unrelated writes
        vq = [vaug_p.tile([128, 4, NH, 65], bf16, name=f"vq{q}") for q in range(4)]
        for q in range(4):
            nc.vector.memset(vq[q][:, :, :, 64:65], 1.0)

        # per-chunk q/k tiles and per-pair attention-output tiles: keeps
        # the subtile dependency tracking precise (a monolithic tile makes
        # late readers wait on the newest writer of ANY region)
        qTc = [qk_p.tile([128, 4, 512], bf16, name=f"qT{c}") for c in range(4)]
        kTc = [qk_p.tile([128, 4, 512], bf16, name=f"kT{c}") for c in range(4)]
        aoTm = [ao_p.tile([128, L], bf16, name=f"aoT{m}") for m in range(4)]

        wvT_sb = wv_p.tile([128, ET, JQ], bf16)           # 8KB/part
        wqkT_sb = wqk_p.tile([128, ET, 2 * JQ], bf16)     # 16KB/part
        woT_sb = wo_p.tile([128, 4, E], bf16)             # 8KB/part

        def half_load(dst, src, cols):
            # split a (et p) input load into two et-halves so compute can
            # start when the first half lands
            for h in range(2):
                nc.sync.dma_start(
                    out=dst[:, 4 * h:4 * h + 4, :],
                    in_=src[512 * h:512 * h + 512, cols].rearrange(
                        "(et p) l -> p et l", p=128
                    ),
                )

        xcs = [xT_p.tile([128, ET, 512], bf16, tag="xc", name=f"xc{c}") for c in range(4)]
        for h in range(2):  # interleaved so the first et-half lands first
            nc.sync.dma_start(
                out=wvT_sb[:, 4 * h:4 * h + 4, :],
                in_=wqkvT_d.ap()[512 * h:512 * h + 512, 2 * JQ:3 * JQ].rearrange(
                    "(et p) j -> p et j", p=128),
            )
            nc.sync.dma_start(
                out=xcs[0][:, 4 * h:4 * h + 4, :],
                in_=xT_d.ap()[512 * h:512 * h + 512, 0:512].rearrange(
                    "(et p) l -> p et l", p=128),
            )
        half_load(wqkT_sb, wqkvT_d.ap(), slice(0, 2 * JQ))

        def dense_ps(pp, nm):
            # heaters get the dedicated 1-bank pool; dense phases borrow the
            # (idle) 2-bank score rotation so back-to-back units pipeline
            if pp is heat_pp:
                return pp.tile([128, 512], f32, tag="hp", name=nm)
            return pp.tile([128, 2, 512], f32, tag="sc", name=nm)[:, 0, :]

        def v_unit(pp, xc, lt, i):
            ps = dense_ps(pp, "vps")
            for et in range(ET):
                nc.tensor.matmul(
                    ps,
                    lhsT=xc[:, et, i * 128:(i + 1) * 128],
                    rhs=wvT_sb[:, et, :],
                    start=(et == 0), stop=(et == ET - 1),
                )
            nc.vector.tensor_copy(
                out=vq[lt // 4][:, lt % 4, :, 0:64],
                in_=ps.rearrange("p (h d) -> p h d", h=NH),
            )

        def qk_unit(pp, xc, jt, c):
            # jt 0..3 = q j-tiles, 4..7 = k j-tiles
            ps = dense_ps(pp, "qkps")
            dst = qTc[c] if jt < 4 else kTc[c]
            for et in range(ET):
                nc.tensor.matmul(
                    ps,
                    lhsT=wqkT_sb[:, et, jt * 128:(jt + 1) * 128],
                    rhs=xc[:, et, :],
                    start=(et == 0), stop=(et == ET - 1),
                )
            nc.vector.tensor_copy(out=dst[:, jt % 4, :], in_=ps)

        def op_unit(pp, lt, ec):
            ps = dense_ps(pp, "opps")
            for jt in range(4):
                nc.tensor.matmul(
                    ps,
                    lhsT=aoTm[jt][:, lt * 128:(lt + 1) * 128],
                    rhs=woT_sb[:, jt, ec * 512:(ec + 1) * 512],
                    start=(jt == 0), stop=(jt == 3),
                )
            yt = y_p.tile([128, 512], f32, tag="y")
            nc.vector.tensor_copy(out=yt, in_=ps)
            nc.sync.dma_start(
                out=y_d.ap()[lt * 128:(lt + 1) * 128, ec * 512:(ec + 1) * 512],
                in_=yt,
            )

        # ---------------- P0a: v lt0-3 + qk chunk 0 (minimum dense prefix) ---
        nunit = 0

        def alt_pool():
            nonlocal nunit
            nunit += 1
            return heat_pp if nunit % 2 else sc_pp

        half_load(xcs[1], xT_d.ap(), slice(512, 1024))
        nc.sync.dma_start(
            out=diag2_sb, in_=diag2_d.ap().rearrange("p (a k) -> p a k", a=2)
        )
        for i in range(4):
            v_unit(alt_pool(), xcs[0], i, i)
        half_load(xcs[2], xT_d.ap(), slice(1024, 1536))
        half_load(xcs[3], xT_d.ap(), slice(1536, 2048))
        nc.sync.dma_start(
            out=woT_sb, in_=woT_d.ap().rearrange("(jt p) e -> p jt e", p=128)
        )
        for jt in range(8):
            qk_unit(alt_pool(), xcs[0], jt, 0)

        # ---------------- attention ----------------
        def attn_chunk(m, c, heaters, period, last=False):
            """Head pair m (heads 2m, 2m+1), lq chunk c (cols 512c..512c+511)."""
            ntiles = 4 * c + 4
            pv_A = pv_pp.tile([65, 512], f32, tag="pv", name="pvA")
            pv_B = pv_pp.tile([65, 512], f32, tag="pv", name="pvB")
            pvs = (pv_A, pv_B)
            pending = None

            def emit_pv(pend, last):
                pe, off, t = pend
                for hi in (0, 1):
                    nc.tensor.matmul(
                        pvs[hi][:, off:512],
                        lhsT=vq[t // 4][:, t % 4, 2 * m + hi, :],
                        rhs=pe[:, hi, off:512],
                        start=(t == 0), stop=last,
                        skip_group_check=True,
                    )

            for t in range(ntiles):
                j = t - 4 * c
                off = 128 * j if j > 0 else 0
                sc = sc_pp.tile([128, 2, 512], f32, tag="sc", name="sc")
                for hi, po in ((0, 0), (1, 64)):
                    nc.tensor.matmul(
                        sc[:, hi, off:512],
                        lhsT=kTc[t // 4][po:po + 64, m,
                                         (t % 4) * 128:(t % 4) * 128 + 128],
                        rhs=qTc[c][po:po + 64, m, off:512],
                        start=True, stop=True,
                    )
                pe = pe_p.tile([128, 2, 512], bf16, tag="pe")
                nc.scalar.activation(
                    out=pe[:, :, off:512], in_=sc[:, :, off:512], func=Exp, scale=0.125,
                )
                if j >= 0:  # diagonal block: zero lk > lq
                    nc.vector.tensor_mul(
                        out=pe[:, :, off:off + 128],
                        in0=pe[:, :, off:off + 128],
                        in1=diag2_sb,
                    )
                if pending is not None:
                    emit_pv(pending, last=False)
                pending = (pe, off, t)
                if t % period == period - 1 and heaters:
                    heaters.pop(0)()
            emit_pv(pending, last=True)

            # epilogue: evacuate PV accumulators, then reciprocal of the
            # ones-column sums via DRAM respread across 128 lanes (stride-0
            # partition reads are only legal from DRAM), normalize (bf16)
            aoU = aou_p.tile([65, 1024], f32, tag="aou")
            nc.vector.tensor_copy(out=aoU[:, 0:512], in_=pv_A)
            nc.vector.tensor_copy(out=aoU[:, 512:1024], in_=pv_B)
            rcb = rcb_p.tile([64, 1024], f32, tag="rcb")
            rcd = rcd_p.tile([1, 1024], f32, tag="rcd")
            nc.sync.dma_start(out=rcd, in_=aoU[64:65, :])
            rc8 = rc_p.tile([128, 8], f32, tag="rc8")
            nc.sync.dma_start(out=rc8, in_=rcd.rearrange("o (p k) -> (o p) k", p=128))
            nc.vector.reciprocal(out=rc8, in_=rc8)
            rcd2 = rcd_p.tile([1, 1024], f32, tag="rcd2")
            nc.sync.dma_start(out=rcd2.rearrange("o (p k) -> (o p) k", p=128), in_=rc8)
            nc.sync.dma_start(out=rcb, in_=rcd2.to_broadcast((64, 1024)))
            for hi, po in ((0, 0), (1, 64)):
                nc.vector.tensor_mul(
                    out=aoTm[m][po:po + 64, c * 512:(c + 1) * 512],
                    in0=aoU[0:64, hi * 512:hi * 512 + 512],
                    in1=rcb[:, hi * 512:hi * 512 + 512],
                )

        # Phase driver: chunk-c attention for all pairs, carrying the dense
        # units needed by LATER phases as heaters (qk chunk c+1, v quadrant
        # c+1, then the output projection).  Every phase stays PE-bound, so
        # the ACT exp stream hides under dense matmuls instead of pacing.
        def run_phase(c, heaters, period, last=False):
            for m in range(4):
                attn_chunk(m, c, heaters, period, last=last and m == 3)
            while heaters:
                heaters.pop(0)()

        run_phase(0, (
            [lambda jt=jt: qk_unit(heat_pp, xcs[1], jt, 1) for jt in range(8)]
            + [lambda i=i: v_unit(heat_pp, xcs[1], 4 + i, i) for i in range(4)]
        ), period=1)
        run_phase(1, (
            [lambda jt=jt: qk_unit(heat_pp, xcs[2], jt, 2) for jt in range(8)]
            + [lambda i=i: v_unit(heat_pp, xcs[2], 8 + i, i) for i in range(4)]
        ), period=2)
        run_phase(2, (
            [lambda jt=jt: qk_unit(heat_pp, xcs[3], jt, 3) for jt in range(8)]
            + [lambda i=i: v_unit(heat_pp, xcs[3], 12 + i, i) for i in range(4)]
        ), period=4)
        # period 4: 16 of the 24 outproj units interleave into the (PE-bound)
        # c3 phase; the remaining 8 drain right after the last pair's PV —
        # i.e. inside the final denominator-chain window that was previously
        # pure PE idle (and dropped the clock to K=4/8 for the tail).
        run_phase(3, (
            [lambda lt=lt, ec=ec: op_unit(heat_pp, lt, ec)
             for lt in range(12) for ec in range(2)]
        ), period=4)

        # tail: outproj rows 1536..2047
        for lt in range(12, LT):
            for ec in range(2):
                op_unit(alt_pool(), lt, ec)

    nc.compile()
    return nc


def make_in_maps(x, w_qkv, wo):
    """Host-side sharding: 8 cores = (batch b=c//2, head-group g=c%2)."""
    import ml_dtypes
    bf16 = ml_dtypes.bfloat16

    x = np.asarray(x, dtype=np.float32)
    w_qkv = np.asarray(w_qkv, dtype=np.float32)
    wo = np.asarray(wo, dtype=np.float32)
    tri = np.triu(np.ones((128, 128), np.float32))
    diag2 = np.concatenate([tri, tri], axis=1).astype(bf16)
    in_maps = []
    for c in range(8):
        b, g = c // 2, c % 2
        js = slice(g * JQ, (g + 1) * JQ)
        wq = w_qkv[0:E][js]
        wk = w_qkv[E:2 * E][js]
        wv = w_qkv[2 * E:3 * E][js]
        in_maps.append({
            "xT": np.ascontiguousarray(x[b].T).astype(bf16),
            "wqkvT": np.ascontiguousarray(np.concatenate([wq, wk, wv], 0).T).astype(bf16),
            "woT": np.ascontiguousarray(wo[:, js].T).astype(bf16),
            "diag2": diag2,
        })
    return in_maps


def _get_nc():
    if "nc" not in _CACHE:
        _CACHE["nc"] = build_nc()
    return _CACHE["nc"]


def kernel(x, mask, w_qkv, wo, _trace=False, _trace_kwargs=None):
    from concourse.bass_utils import run_bass_kernel_spmd

    nc = _get_nc()
    in_maps = make_in_maps(x, w_qkv, wo)
    res = run_bass_kernel_spmd(
        nc, in_maps, core_ids=list(range(8)),
        trace=_trace, **(_trace_kwargs or {}),
    )
    _CACHE["last_results"] = res
    y = np.stack([res.results[2 * b]["y"] + res.results[2 * b + 1]["y"] for b in range(4)])
    return y.astype(np.float32)
